# revision 1
# baseline (speedup 1.0000x reference)
"""GCN 2-layer encoder on 8 Trainium2 NeuronCores (Bass/Tile).

Sharding: nodes in 8 contiguous blocks of 12500 (dst-owner aggregates).
Per layer per core: h = x_local @ W (PE), y = dinv*h published to a
per-quarter AllGather'd table; edge messages fetched with dma_gather
(int16 idx => 4 node-quarters per core-slice, tables <= 25600 rows);
aggregation = one-hot (tensor_scalar is_equal) matmuls accumulating in
PSUM dst-windows of 128 nodes, window-blocks of 12 windows double-
buffered across 6 PSUM banks; self-loop added at window flush.

All CPU-side work is integer graph partitioning/relabeling (sharding);
every float op runs on device, f32 end to end.
"""
import os
import sys

sys.path.insert(0, "/opt/trn_rl_repo")
import numpy as np

import concourse.tile as tile
from concourse import bacc, mybir, library_config
from concourse.bass_utils import run_bass_kernel_spmd

N_NODES = 100000
N_CORES = 8
S = N_NODES // N_CORES          # 12500 nodes per core
D = 128
NW = (S + 127) // 128           # 98 dst windows per core
WPT = 12                        # windows per window-block
NWB = (NW + WPT - 1) // WPT     # 9 window-blocks
QB = [0, 3200, 6400, 9472, S]   # quarter boundaries (tile-aligned starts)
QLEN = [QB[i + 1] - QB[i] for i in range(4)]
CALL = 1024                     # rows per dma_gather (HW-safe max)
f32 = mybir.dt.float32
i16 = mybir.dt.int16


def _quarter_of(m):
    q = np.searchsorted(np.array(QB[1:]), m, side="right")
    return q


def _prep(edge_index):
    """Integer-only graph partitioning -> common SPMD schedule + per-core
    idx/dst arrays. Returns (sched, per_core, deg)."""
    src = np.asarray(edge_index[0], dtype=np.int64)
    dst = np.asarray(edge_index[1], dtype=np.int64)
    deg = np.bincount(dst, minlength=N_NODES).astype(np.int64) + 1

    core = dst // S
    md = dst % S
    w = md // 128                       # window within core
    wb = w // WPT                       # window block
    ms = src % S
    q = _quarter_of(ms)                 # src quarter
    cs = src // S
    idx16 = cs * np.array(QLEN)[q] + (ms - np.array(QB)[q])  # row in table_q

    # per (core, wb, q) segment, edges sorted by (dst, src)
    segs_edges = {}
    order = np.lexsort((src, dst, q, wb, core))
    coreo, wbo, qo = core[order], wb[order], q[order]
    mdo, idxo = md[order], idx16[order]
    key = ((coreo * NWB + wbo) * 4 + qo)
    bounds = np.flatnonzero(np.diff(key)) + 1
    starts = np.concatenate([[0], bounds])
    ends = np.concatenate([bounds, [len(key)]])
    for s0, e0 in zip(starts, ends):
        segs_edges[int(key[s0])] = (mdo[s0:e0], idxo[s0:e0])

    # common segment lengths
    seg_list = [(b, qq) for b in range(NWB) for qq in range(4)]
    L = {}
    for (b, qq) in seg_list:
        mx = 0
        for c in range(N_CORES):
            k = (c * NWB + b) * 4 + qq
            if k in segs_edges:
                mx = max(mx, len(segs_edges[k][0]))
        L[(b, qq)] = max(128, ((mx + 127) // 128) * 128)

    tot_slots = sum(L.values())
    n_chunk_tot = tot_slots // 128

    # per-core padded arrays: gather idx (slot-major) and dstm per slot
    gidx_flat = np.zeros((N_CORES, tot_slots), dtype=np.int16)
    dstm_flat = np.full((N_CORES, tot_slots), -100000.0, dtype=np.float32)
    seg_base = {}
    off = 0
    for (b, qq) in seg_list:
        seg_base[(b, qq)] = off
        for c in range(N_CORES):
            k = (c * NWB + b) * 4 + qq
            if k in segs_edges:
                mdl, idxl = segs_edges[k]
                n = len(mdl)
                gidx_flat[c, off:off + n] = idxl.astype(np.int16)
                dstm_flat[c, off:off + n] = mdl.astype(np.float32)
        off += L[(b, qq)]

    # chunk -> union of touched windows across cores; op list
    # ops[i] = (seg b,q, chunk j, window w, start, stop)
    ops = []
    first_op = {}
    last_op = {}
    for (b, qq) in seg_list:
        base = seg_base[(b, qq)]
        nch = L[(b, qq)] // 128
        for j in range(nch):
            sl = slice(base + 128 * j, base + 128 * (j + 1))
            vals = dstm_flat[:, sl]
            real = vals >= 0
            if not real.any():
                continue
            wins = np.unique((vals[real] // 128).astype(np.int64))
            for wv in wins:
                wv = int(wv)
                oi = len(ops)
                ops.append([b, qq, j, wv, False, False])
                if (b, wv) not in first_op:
                    first_op[(b, wv)] = oi
                last_op[(b, wv)] = oi
    # start/stop must be once per PSUM *bank* per window-block: start=True
    # clears the whole bank (slot-3-only windows survived on HW), so flag
    # only the first/last op among the 4 window slots sharing a bank.
    first_bk, last_bk = {}, {}
    for oi, (b, qq, j, wv, _, _) in enumerate(ops):
        bk = (b, (wv - b * WPT) // 4)
        if bk not in first_bk:
            first_bk[bk] = oi
        last_bk[bk] = oi
    for oi in first_bk.values():
        ops[oi][4] = True
    for oi in last_bk.values():
        ops[oi][5] = True

    # dstm per chunk column (device compare: (iota - dstm) == -128*w)
    dstmT = np.empty((N_CORES, 128, n_chunk_tot), dtype=np.float32)
    for jg in range(n_chunk_tot):
        dstmT[:, :, jg] = dstm_flat[:, 128 * jg:128 * (jg + 1)]

    # wrap gather idx: [128, tot/16], idx i at (i%16, i//16), 8x replicated
    gidx_w = np.empty((N_CORES, 128, tot_slots // 16), dtype=np.int16)
    for c in range(N_CORES):
        a = gidx_flat[c].reshape(-1, 16).T        # [16, tot/16]
        gidx_w[c] = np.tile(a, (8, 1))

    sched = {
        "L": L, "seg_list": seg_list, "seg_base": seg_base,
        "ops": ops, "tot_slots": tot_slots, "n_chunk_tot": n_chunk_tot,
        "windows_per_wb": [min(NW - b * WPT, WPT) for b in range(NWB)],
        "first_op": first_op, "last_op": last_op,
    }
    return sched, gidx_w, dstmT, deg


def _build(sched, repeat=1, debug=False, hoist_collectives=False):
    nc = bacc.Bacc("TRN2", target_bir_lowering=False, debug=False,
                   num_devices=N_CORES)
    NCOL = NW * 128                                   # 12544 padded nodes
    xT = nc.dram_tensor("xT", [128, NCOL], f32, kind="ExternalInput")
    W1 = nc.dram_tensor("W1", [128, 128], f32, kind="ExternalInput")
    W2 = nc.dram_tensor("W2", [128, 128], f32, kind="ExternalInput")
    b1b = nc.dram_tensor("b1b", [128, 128], f32, kind="ExternalInput")
    b2b = nc.dram_tensor("b2b", [128, 128], f32, kind="ExternalInput")
    degi = nc.dram_tensor("degi", [128, NW], f32, kind="ExternalInput")
    iotad = nc.dram_tensor("iotad", [128, 128], f32, kind="ExternalInput")
    identd = nc.dram_tensor("identd", [128, 128], f32, kind="ExternalInput")
    gidx = nc.dram_tensor("gidx", [128, sched["tot_slots"] // 16], i16,
                          kind="ExternalInput")
    dstmT = nc.dram_tensor("dstmT", [128, sched["n_chunk_tot"]], f32,
                           kind="ExternalInput")
    out = nc.dram_tensor("out", [S, D], f32, kind="ExternalOutput")
    if debug:
        dbg_y1 = nc.dram_tensor("dbg_y1", [128, NW * 128], f32,
                                kind="ExternalOutput")
        dbg_b1 = nc.dram_tensor("dbg_b1", [128, NW * 128], f32,
                                kind="ExternalOutput")

    y_slice = nc.dram_tensor("y_slice", [S, D], f32)
    tables = [nc.dram_tensor(f"table{qq}", [N_CORES * QLEN[qq], D], f32,
                             addr_space="Shared") for qq in range(4)]

    seg_list, L, seg_base = sched["seg_list"], sched["L"], sched["seg_base"]
    ops, wpwb = sched["ops"], sched["windows_per_wb"]

    # group ops by segment for emission order
    ops_by_seg = {sk: [] for sk in seg_list}
    for op in ops:
        ops_by_seg[(op[0], op[1])].append(op)

    with tile.TileContext(nc) as tc:
        with (
            tc.tile_pool(name="cst", bufs=1) as cst,
            tc.tile_pool(name="big", bufs=1) as big,
            tc.tile_pool(name="st", bufs=3) as stp,
            tc.tile_pool(name="oh", bufs=4) as ohp,
            tc.tile_pool(name="bank", bufs=1, space="PSUM") as bankp,
            tc.tile_pool(name="php", bufs=2, space="PSUM") as php,
            tc.tile_pool(name="tmp", bufs=3) as tmp,
        ):
            nc.gpsimd.load_library(library_config.mlp)

            xT_sb = cst.tile([128, NCOL], f32, tag="xT")
            W1_sb = cst.tile([128, 128], f32, tag="W1")
            W2_sb = cst.tile([128, 128], f32, tag="W2")
            b1_sb = cst.tile([128, 128], f32, tag="b1")
            b2_sb = cst.tile([128, 128], f32, tag="b2")
            deg_sb = cst.tile([128, NW], f32, tag="deg")
            dinv_sb = cst.tile([128, NW], f32, tag="dinv")
            iota_sb = cst.tile([128, 128], f32, tag="iota")
            id_sb = cst.tile([128, 128], f32, tag="ident")
            gidx_sb = cst.tile([128, sched["tot_slots"] // 16], i16, tag="gx")
            dstm_sb = cst.tile([128, sched["n_chunk_tot"]], f32, tag="dm")
            A = big.tile([128, NCOL], f32, tag="A")   # y_local (self-loop)
            B = big.tile([128, NCOL], f32, tag="B")   # aggregation acc
            banks = [bankp.tile([128, 512], f32, tag=f"bk{i}",
                                name=f"bank{i}")
                     for i in range(6)]

            nc.sync.dma_start(xT_sb[:], xT[:])
            nc.sync.dma_start(W1_sb[:], W1[:])
            nc.sync.dma_start(W2_sb[:], W2[:])
            nc.sync.dma_start(b1_sb[:], b1b[:])
            nc.sync.dma_start(b2_sb[:], b2b[:])
            nc.sync.dma_start(deg_sb[:], degi[:])
            nc.sync.dma_start(iota_sb[:], iotad[:])
            nc.sync.dma_start(id_sb[:], identd[:])
            nc.sync.dma_start(gidx_sb[:], gidx[:])
            nc.sync.dma_start(dstm_sb[:], dstmT[:])
            nc.vector.reciprocal(dinv_sb[:], deg_sb[:])
            nc.scalar.activation(dinv_sb[:], dinv_sb[:],
                                 mybir.ActivationFunctionType.Sqrt)

            def publish_collectives():
                for qq in range(4):
                    nc.gpsimd.collective_compute(
                        "AllGather", mybir.AluOpType.bypass,
                        replica_groups=[list(range(N_CORES))],
                        ins=[y_slice.ap()[QB[qq]:QB[qq + 1], :].opt()],
                        outs=[tables[qq].ap().opt()])

            def publish_quarters(layer):
                """DMA A (y, node-major [p, t*128+f]) quarter slices to
                y_slice rows, then per-quarter AllGather into tables."""
                for qq in range(4):
                    r0, r1 = QB[qq], QB[qq + 1]
                    t0, p0 = r0 // 128, r0 % 128
                    t1, p1 = r1 // 128, r1 % 128
                    assert p0 == 0
                    tf = t1 if p1 else t1
                    if t1 > t0:
                        nc.sync.dma_start(
                            y_slice.ap()[r0:128 * t1, :].rearrange(
                                "(t p) f -> p t f", p=128),
                            A[:, 128 * t0:128 * t1].rearrange(
                                "p (t f) -> p t f", f=128))
                    if p1:
                        nc.sync.dma_start(
                            y_slice.ap()[128 * t1:r1, :],
                            A[0:p1, 128 * t1:128 * (t1 + 1)])
                if not hoist_collectives:
                    publish_collectives()

            def aggregate(layer):
                """Gather + one-hot matmul accumulate + flush into B."""
                for b in range(NWB):
                    nwin = wpwb[b]
                    for qq in range(4):
                        base = seg_base[(b, qq)]
                        Lseg = L[(b, qq)]
                        ncalls = (Lseg + CALL - 1) // CALL
                        stages = []
                        for k in range(ncalls):
                            cl = min(CALL, Lseg - CALL * k)
                            stg = stp.tile([128, CALL // 128, 128], f32,
                                           tag="stg")
                            nc.gpsimd.dma_gather(
                                stg[:, :cl // 128, :], tables[qq].ap(),
                                gidx_sb[:, (base + CALL * k) // 16:
                                        (base + CALL * k + cl) // 16],
                                cl, cl, 128)
                            stages.append(stg)
                        for op in ops_by_seg[(b, qq)]:
                            _, _, j, wv, st_f, sp_f = op
                            jg = (base // 128) + j
                            k, jc = j // (CALL // 128), j % (CALL // 128)
                            S_t = ohp.tile([128, 128], f32, tag="S")
                            nc.vector.tensor_scalar(
                                S_t[:], iota_sb[:],
                                dstm_sb[:, jg:jg + 1], float(-128.0 * wv),
                                op0=mybir.AluOpType.subtract,
                                op1=mybir.AluOpType.is_equal)
                            wl = wv - b * WPT
                            bank = banks[(b % 2) * 3 + wl // 4]
                            bsl = bank[:, 128 * (wl % 4):128 * (wl % 4 + 1)]
                            nc.tensor.matmul(
                                bsl, lhsT=S_t[:], rhs=stages[k][:, jc, :],
                                start=st_f, stop=sp_f)
                    # flush this window block
                    for wl in range(nwin):
                        wv = b * WPT + wl
                        csl = slice(128 * wv, 128 * (wv + 1))
                        bank = banks[(b % 2) * 3 + wl // 4]
                        bsl = bank[:, 128 * (wl % 4):128 * (wl % 4 + 1)]
                        if (b, wv) in sched["first_op"]:
                            nc.vector.tensor_tensor(
                                B[:, csl], bsl, A[:, csl],
                                op=mybir.AluOpType.add)
                        else:
                            nc.vector.tensor_copy(B[:, csl], A[:, csl])

            if hoist_collectives:
                publish_collectives()
                publish_collectives()
            loop_cm = tc.For_i(0, repeat, 1) if repeat > 1 else None
            if loop_cm is not None:
                loop_cm.__enter__()

            # ---------------- layer 1 ----------------
            for t in range(NW):
                h_ps = php.tile([128, 128], f32, tag="php")
                nc.tensor.matmul(h_ps[:], lhsT=xT_sb[:, 128 * t:128 * (t + 1)],
                                 rhs=W1_sb[:], start=True, stop=True)
                nc.vector.tensor_scalar(A[:, 128 * t:128 * (t + 1)], h_ps[:],
                                        dinv_sb[:, t:t + 1], None,
                                        op0=mybir.AluOpType.mult)
            if debug:
                nc.sync.dma_start(dbg_y1.ap(), A[:])
            publish_quarters(0)
            aggregate(0)
            if debug:
                nc.sync.dma_start(dbg_b1.ap(), B[:])
            # out1 = B*dinv + b1 ; z = relu(out1) -> A
            for t in range(NW):
                csl = slice(128 * t, 128 * (t + 1))
                nc.vector.tensor_scalar(B[:, csl], B[:, csl],
                                        dinv_sb[:, t:t + 1], None,
                                        op0=mybir.AluOpType.mult)
                nc.vector.tensor_tensor(B[:, csl], B[:, csl], b1_sb[:],
                                        op=mybir.AluOpType.add)
            nc.scalar.activation(A[:], B[:],
                                 mybir.ActivationFunctionType.Relu)

            # ---------------- layer 2 ----------------
            for t in range(NW):
                csl = slice(128 * t, 128 * (t + 1))
                t_ps = php.tile([128, 128], f32, tag="php")
                nc.tensor.transpose(t_ps[:], A[:, csl], id_sb[:])
                zT = tmp.tile([128, 128], f32, tag="zT")
                nc.vector.tensor_copy(zT[:], t_ps[:])
                h_ps = php.tile([128, 128], f32, tag="php")
                nc.tensor.matmul(h_ps[:], lhsT=zT[:], rhs=W2_sb[:],
                                 start=True, stop=True)
                nc.vector.tensor_scalar(A[:, csl], h_ps[:],
                                        dinv_sb[:, t:t + 1], None,
                                        op0=mybir.AluOpType.mult)
            publish_quarters(1)
            aggregate(1)
            for t in range(NW):
                csl = slice(128 * t, 128 * (t + 1))
                nc.vector.tensor_scalar(B[:, csl], B[:, csl],
                                        dinv_sb[:, t:t + 1], None,
                                        op0=mybir.AluOpType.mult)
                nc.vector.tensor_tensor(B[:, csl], B[:, csl], b2_sb[:],
                                        op=mybir.AluOpType.add)
            # write output rows 0..12500
            nc.sync.dma_start(
                out.ap()[0:128 * 97, :].rearrange("(t p) f -> p t f", p=128),
                B[:, 0:128 * 97].rearrange("p (t f) -> p t f", f=128))
            nc.sync.dma_start(out.ap()[128 * 97:S, :],
                              B[0:S - 128 * 97, 128 * 97:128 * 98])

            if loop_cm is not None:
                loop_cm.__exit__(None, None, None)

    nc.compile()
    return nc


def _make_in_maps(x, W1, b1, W2, b2, sched, gidx_w, dstmT, deg):
    NCOL = NW * 128
    iota = np.broadcast_to(np.arange(128, dtype=np.float32),
                           (128, 128)).copy()
    ident = np.eye(128, dtype=np.float32)
    in_maps = []
    for c in range(N_CORES):
        xs = x[S * c:S * (c + 1)].astype(np.float32)
        xT = np.zeros((128, NCOL), np.float32)
        xT[:, :S] = xs.T
        # reorder columns to node-major [p=n%128, t=n//128] layout:
        # xT column layout IS n along free dim; device expects col=t*128+?:
        # lhsT tile t = xT[:, 128t:128(t+1)] = features x rows ✓ already.
        degc = deg[S * c:S * (c + 1)].astype(np.float32)
        degp = np.ones(NCOL, np.float32)
        degp[:S] = degc
        # node n -> [n%128, n//128]
        deg_pc = degp.reshape(NW, 128).T.copy()
        in_maps.append({
            "xT": xT,
            "W1": W1.astype(np.float32), "W2": W2.astype(np.float32),
            "b1b": np.broadcast_to(b1.astype(np.float32), (128, 128)).copy(),
            "b2b": np.broadcast_to(b2.astype(np.float32), (128, 128)).copy(),
            "degi": deg_pc, "iotad": iota, "identd": ident,
            "gidx": gidx_w[c], "dstmT": dstmT[c],
        })
    return in_maps


def kernel(x, edge_index, W1, b1, W2, b2):
    sched, gidx_w, dstmT, deg = _prep(edge_index)
    nc = _build(sched, repeat=int(os.environ.get("KERNEL_REPEAT", "1")))
    in_maps = _make_in_maps(x, W1, b1, W2, b2, sched, gidx_w, dstmT, deg)
    res = run_bass_kernel_spmd(nc, in_maps, core_ids=list(range(N_CORES)))
    return np.concatenate([res.results[c]["out"] for c in range(N_CORES)], 0)



# revision 7
# speedup vs baseline: 4.9093x; 4.9093x over previous
"""GCN 2-layer encoder on 8 Trainium2 NeuronCores — zero-collective design.

Every core redundantly computes the cheap dense transforms for ALL nodes
(y1 = dinv*(x@W1), zT = relu-agg, y2 = dinv*(z@W2)) into LOCAL bf16 HBM
gather tables, so no AllGather is ever needed (collectives cost ~5ms each
on this runtime when awaited).  Layer-1 aggregation is replicated over all
100352 padded nodes (one-hot matmul segment-sum, gather-bandwidth bound);
layer-2 aggregation only covers the core's own 12500 dst nodes.

Messages are bf16 (halves gather bytes; PSUM accumulates f32).  The
one-hot S_t fuses dinv[dst] via tensor_scalar(is_equal, mult) with
per-op dstmS / per-chunk dd streamed from HBM per window-block.
"""
import os
import sys

sys.path.insert(0, "/opt/trn_rl_repo")
import numpy as np
import ml_dtypes

import concourse.tile as tile
from concourse import bacc, mybir, library_config
from concourse.bass_utils import run_bass_kernel_spmd

N_NODES = 100000
N_CORES = 8
S = N_NODES // N_CORES          # 12500 own nodes per core
D = 128
NPAD = 100352                   # 784 * 128
NWG = NPAD // 128               # 784 global dst windows (layer 1)
WPT = 12                        # windows per block (3 PSUM banks x 4)
NWBG = (NWG + WPT - 1) // WPT   # 66 global window blocks
QL = 25088                      # quarter length (196*128), int16-safe
QBG = [0, QL, 2 * QL, 3 * QL, 4 * QL]
NW2 = (S + 127) // 128          # 98 own dst windows (layer 2)
NWB2 = (NW2 + WPT - 1) // WPT   # 9
CALL = int(os.environ.get("KERNEL_CALL", "1024"))
f32 = mybir.dt.float32
bf16 = mybir.dt.bfloat16
i16 = mybir.dt.int16


def _pad128(n):
    return max(128, ((n + 127) // 128) * 128)


def _wrap_idx(gidx_flat):
    """[slots] int16 -> [128, slots/16] wrapped+replicated for dma_gather."""
    a = gidx_flat.reshape(-1, 16).T
    return np.tile(a, (8, 1)).copy()


def _sched_edges(src, dst, dinv, nwb, n_seg_q, dst_base=0, with_dd=True):
    """Common scheduling: segment edges by (window-block of dst, quarter of
    src), pad each segment to x128, emit chunk/op tables.

    dst here is already local (dst_base subtracted).  Returns dict with
    gidx [slots], dstm [128, nch] (local dst value per slot), per-op arrays,
    and op metadata list (b, q, k, jc, jl, wl, start, stop, oi).
    """
    w = dst // 128
    b = w // WPT
    q = src // QL
    if os.environ.get("SORT_DST"):
        order = np.lexsort((src, dst, q, b))
    else:
        order = np.lexsort((src, w, q, b))
    so, do, qo, bo = src[order], dst[order], q[order], b[order]
    segkey = bo * 4 + qo
    n = len(so)
    bounds = np.flatnonzero(np.diff(segkey)) + 1
    starts = np.concatenate([[0], bounds])
    ends = np.concatenate([bounds, [n]])
    seg_edges = {int(segkey[s0]): (s0, e0) for s0, e0 in zip(starts, ends)}

    seg_list = [(bb, qq) for bb in range(nwb) for qq in range(4)]
    L = {}
    for (bb, qq) in seg_list:
        k = bb * 4 + qq
        ln = seg_edges[k][1] - seg_edges[k][0] if k in seg_edges else 0
        L[(bb, qq)] = _pad128(ln) if ln else 0
    tot = sum(L.values())
    gidx = np.zeros(tot, np.int16)
    dstv = np.full(tot, -1.0e6, np.float64)
    ddv = np.zeros(tot, np.float64)
    seg_base = {}
    off = 0
    for (bb, qq) in seg_list:
        seg_base[(bb, qq)] = off
        k = bb * 4 + qq
        if k in seg_edges:
            s0, e0 = seg_edges[k]
            m = e0 - s0
            gidx[off:off + m] = (so[s0:e0] - QBG[qo[s0]]).astype(np.int16)
            dstv[off:off + m] = do[s0:e0]
            if with_dd:
                ddv[off:off + m] = dinv[do[s0:e0] + dst_base]
        off += L[(bb, qq)]

    nch = tot // 128
    slot_ch = np.arange(tot) // 128
    real = dstv >= 0
    wl_slot = np.where(real, (dstv // 128).astype(np.int64), 0)
    # ops: unique (chunk, window) among real slots
    opkey = slot_ch * NWG + wl_slot
    op_keys = np.unique(opkey[real])
    n_ops = len(op_keys)
    op_of_slot = np.searchsorted(op_keys, opkey)
    # per-op dstm (dst - 128*w for slots in that op's window, else -1000)
    dstmS = np.full((n_ops, 128), -1000.0, np.float32)
    sl = np.flatnonzero(real)
    dstmS[op_of_slot[sl], sl % 128] = (dstv[sl] - 128.0 * wl_slot[sl])
    dstmS = np.ascontiguousarray(dstmS.T)             # [128, n_ops]
    ddT = np.zeros((nch, 128), np.float32)
    ddT[slot_ch[sl], sl % 128] = ddv[sl]
    ddT = np.ascontiguousarray(ddT.T)                 # [128, nch]
    dstmC = np.full((nch, 128), -1.0e6, np.float32)
    dstmC[slot_ch[sl], sl % 128] = dstv[sl]
    dstmC = np.ascontiguousarray(dstmC.T)             # [128, nch]

    # op metadata in emission order (seg-major)
    op_ch = op_keys // NWG
    op_w = op_keys % NWG
    ops = []
    first_bk, last_bk = {}, {}
    for oi in range(n_ops):
        j = int(op_ch[oi])
        wv = int(op_w[oi])
        base = j * 128
        # find seg of this chunk
        bb = int(wv // WPT)
        qq = None
        for qx in range(4):
            sb = seg_base[(bb, qx)]
            if sb <= base < sb + L[(bb, qx)]:
                qq = qx
                break
        assert qq is not None, (j, wv, bb)
        jseg = (base - seg_base[(bb, qq)]) // 128
        ops.append([bb, qq, jseg, j, wv, oi, False, False])
        bk = (bb, (wv - bb * WPT) // 4)
        if bk not in first_bk:
            first_bk[bk] = len(ops) - 1
        last_bk[bk] = len(ops) - 1
    for i in first_bk.values():
        ops[i][6] = True
    for i in last_bk.values():
        ops[i][7] = True
    ops_by_seg = {sk: [] for sk in seg_list}
    for op in ops:
        ops_by_seg[(op[0], op[1])].append(op)
    touched_w = set(int(x) for x in np.unique(op_w))
    return {
        "L": L, "seg_base": seg_base, "seg_list": seg_list,
        "tot": tot, "nch": nch, "n_ops": n_ops,
        "gidx": gidx, "dstmS": dstmS, "ddT": ddT, "dstmC": dstmC,
        "ops_by_seg": ops_by_seg, "touched_w": touched_w,
    }


def _prep(edge_index):
    """Returns (sched_l1, scheds_l2, dinv). sched_l1 is common to all
    cores; scheds_l2 is per-core with common shapes/op-structure."""
    src = np.asarray(edge_index[0], dtype=np.int64)
    dst = np.asarray(edge_index[1], dtype=np.int64)
    deg = (np.bincount(dst, minlength=N_NODES) + 1).astype(np.float64)
    dinv = (1.0 / np.sqrt(deg)).astype(np.float64)

    loop = np.arange(N_NODES, dtype=np.int64)
    src_all = np.concatenate([src, loop])
    dst_all = np.concatenate([dst, loop])

    s1 = _sched_edges(src_all, dst_all, dinv, NWBG, 4)

    # layer 2: per-core own dst, common padding/op structure
    core = dst_all // S
    percore = []
    for c in range(N_CORES):
        m = core == c
        percore.append(_sched_edges(src_all[m], dst_all[m] - c * S, dinv,
                                    NWB2, 4, dst_base=c * S, with_dd=False))
    # unify L across cores, rebuild with common layout
    Lc = {}
    for sk in percore[0]["L"]:
        Lc[sk] = max(p["L"][sk] for p in percore)
    totc = sum(Lc.values())
    nchc = totc // 128
    seg_base = {}
    off = 0
    for sk in percore[0]["seg_list"]:
        seg_base[sk] = off
        off += Lc[sk]
    # re-embed each core's slot arrays into the common layout
    gidx2 = np.zeros((N_CORES, totc), np.int16)
    dstm2 = np.full((N_CORES, 128, nchc), -1.0e6, np.float32)
    opset = set()
    for c in range(N_CORES):
        p = percore[c]
        for sk in p["seg_list"]:
            sb_c, sb_u = p["seg_base"][sk], seg_base[sk]
            ln = p["L"][sk]
            if ln == 0:
                continue
            gidx2[c, sb_u:sb_u + ln] = p["gidx"][sb_c:sb_c + ln]
            for j in range(ln // 128):
                dstm2[c, :, (sb_u // 128) + j] = \
                    p["dstmC"][:, (sb_c // 128) + j]
        # collect op (chunk_global_unified, window) pairs
        for sk in p["seg_list"]:
            for op in p["ops_by_seg"][sk]:
                bb, qq, jseg, _, wv = op[0], op[1], op[2], op[3], op[4]
                ju = (seg_base[(bb, qq)] // 128) + jseg
                opset.add((bb, qq, jseg, ju, wv))
    ops2 = sorted(opset, key=lambda t: (t[0], t[1], t[2], t[4]))
    ops_list = []
    first_bk, last_bk = {}, {}
    for (bb, qq, jseg, ju, wv) in ops2:
        ops_list.append([bb, qq, jseg, ju, wv, None, False, False])
        bk = (bb, (wv - bb * WPT) // 4)
        if bk not in first_bk:
            first_bk[bk] = len(ops_list) - 1
        last_bk[bk] = len(ops_list) - 1
    for i in first_bk.values():
        ops_list[i][6] = True
    for i in last_bk.values():
        ops_list[i][7] = True
    ops_by_seg = {sk: [] for sk in percore[0]["seg_list"]}
    for op in ops_list:
        ops_by_seg[(op[0], op[1])].append(op)
    touched = set()
    for c in range(N_CORES):
        touched |= percore[c]["touched_w"]
    sched_l2 = {
        "L": Lc, "seg_base": seg_base, "tot": totc, "nch": nchc,
        "seg_list": percore[0]["seg_list"],
        "gidx": gidx2, "dstm": dstm2, "ops_by_seg": ops_by_seg,
        "touched_w": touched,
    }
    return s1, sched_l2, dinv.astype(np.float32)


def _build(s1, s2, repeat=1, skip_mm=False, contig_gather=False):
    nc = bacc.Bacc("TRN2", target_bir_lowering=False, debug=False,
                   num_devices=N_CORES, num_swdge_queues=4)
    xT = nc.dram_tensor("xT", [128, NPAD], f32, kind="ExternalInput")
    W1 = nc.dram_tensor("W1", [128, 128], f32, kind="ExternalInput")
    W2b = nc.dram_tensor("W2b", [128, 128], bf16, kind="ExternalInput")
    b1c = nc.dram_tensor("b1c", [128, 1], f32, kind="ExternalInput")
    b2b = nc.dram_tensor("b2b", [128, 128], f32, kind="ExternalInput")
    dinvg = nc.dram_tensor("dinvg", [128, NWG], f32, kind="ExternalInput")
    dinv2 = nc.dram_tensor("dinv2", [128, NW2], f32, kind="ExternalInput")
    iotab = nc.dram_tensor("iotab", [128, 128], bf16, kind="ExternalInput")
    gx1 = nc.dram_tensor("gx1", [128, s1["tot"] // 16], i16,
                         kind="ExternalInput")
    dS1 = nc.dram_tensor("dS1", [128, s1["n_ops"]], f32,
                         kind="ExternalInput")
    dd1 = nc.dram_tensor("dd1", [128, s1["nch"]], f32, kind="ExternalInput")
    gx2 = nc.dram_tensor("gx2", [128, s2["tot"] // 16], i16,
                         kind="ExternalInput")
    dm2 = nc.dram_tensor("dm2", [128, s2["nch"]], f32, kind="ExternalInput")
    out = nc.dram_tensor("out", [S, D], f32, kind="ExternalOutput")

    y1t = [nc.dram_tensor(f"y1t{q}", [QL, 128], bf16) for q in range(4)]
    y2t = [nc.dram_tensor(f"y2t{q}", [QL, 128], bf16) for q in range(4)]
    zT = nc.dram_tensor("zT", [128, NPAD], bf16)

    # per-wb column ranges of dS1/dd1 for streaming
    op_base1, ch_base1 = [], []
    nop_acc = 0
    for bb in range(NWBG):
        op_base1.append(nop_acc)
        nop_acc += sum(len(s1["ops_by_seg"][(bb, qq)]) for qq in range(4))
    op_base1.append(nop_acc)
    assert nop_acc == s1["n_ops"]
    for bb in range(NWBG):
        ch_base1.append(s1["seg_base"][(bb, 0)] // 128)
    ch_base1.append(s1["nch"])

    wpwb_g = [min(NWG - bb * WPT, WPT) for bb in range(NWBG)]
    wpwb_2 = [min(NW2 - bb * WPT, WPT) for bb in range(NWB2)]

    with tile.TileContext(nc) as tc:
        with (
            tc.tile_pool(name="cst", bufs=1) as cst,
            tc.tile_pool(name="blk", bufs=4) as blkp,
            tc.tile_pool(name="ps", bufs=2, space="PSUM") as php,
            tc.tile_pool(name="st", bufs=8) as stp,
            tc.tile_pool(name="gxp", bufs=3) as gxp,
            tc.tile_pool(name="dsp", bufs=2) as dsp,
            tc.tile_pool(name="oh", bufs=4) as ohp,
            tc.tile_pool(name="bank", bufs=1, space="PSUM") as bankp,
            tc.tile_pool(name="fl", bufs=4) as flp,
        ):
            nc.gpsimd.load_library(library_config.mlp)

            W1_sb = cst.tile([128, 128], f32, tag="W1")
            W2_sb = cst.tile([128, 128], bf16, tag="W2")
            b1_sb = cst.tile([128, 1], f32, tag="b1")
            b2_sb = cst.tile([128, 128], f32, tag="b2")
            dinvg_sb = cst.tile([128, NWG], f32, tag="dg")
            dinv2_sb = cst.tile([128, NW2], f32, tag="d2")
            iota_sb = cst.tile([128, 128], bf16, tag="iota")
            dm2_sb = cst.tile([128, s2["nch"]], f32, tag="dm2")
            nc.sync.dma_start(W1_sb[:], W1[:])
            nc.sync.dma_start(W2_sb[:], W2b[:])
            nc.sync.dma_start(b1_sb[:], b1c[:])
            nc.sync.dma_start(b2_sb[:], b2b[:])
            nc.sync.dma_start(dinvg_sb[:], dinvg[:])
            nc.sync.dma_start(dinv2_sb[:], dinv2[:])
            nc.sync.dma_start(iota_sb[:], iotab[:])
            nc.sync.dma_start(dm2_sb[:], dm2[:])
            banks = [bankp.tile([128, 512], f32, tag=f"bk{i}",
                                name=f"bank{i}") for i in range(6)]

            loop_cm = tc.For_i(0, repeat, 1) if repeat > 1 else None
            if loop_cm is not None:
                loop_cm.__enter__()

            # ---- phase A: y1 tables (all nodes) ----
            def build_table(src_dram, w_sb, tabs, src_dt):
                G = 8
                for g0 in range(0, NWG, G):
                    ng = min(G, NWG - g0)
                    xt = blkp.tile([128, G * 128], src_dt, tag="xt")
                    nc.sync.dma_start(
                        xt[:, :ng * 128],
                        src_dram.ap()[:, 128 * g0:128 * (g0 + ng)])
                    yb = blkp.tile([128, G * 128], bf16, tag="yb")
                    for i in range(ng):
                        t = g0 + i
                        ps = php.tile([128, 128], f32, tag="php")
                        nc.tensor.matmul(
                            ps[:], lhsT=xt[:, 128 * i:128 * (i + 1)],
                            rhs=w_sb[:], start=True, stop=True)
                        nc.vector.tensor_scalar(
                            yb[:, 128 * i:128 * (i + 1)], ps[:],
                            dinvg_sb[:, t:t + 1], None,
                            op0=mybir.AluOpType.mult)
                    # DMA out, split at quarter boundaries
                    i = 0
                    while i < ng:
                        t = g0 + i
                        q = (128 * t) // QL
                        nblk = min(ng - i, (QBG[q + 1] - 128 * t) // 128)
                        r0 = 128 * t - QBG[q]
                        nc.sync.dma_start(
                            tabs[q].ap()[r0:r0 + 128 * nblk, :].rearrange(
                                "(t p) f -> p t f", p=128),
                            yb[:, 128 * i:128 * (i + nblk)].rearrange(
                                "p (t f) -> p t f", f=128))
                        i += nblk

            build_table(xT, W1_sb, y1t, f32)

            # ---- phase B: layer-1 aggregation over all nodes -> zT ----
            def aggregate(sched, tabs, gx_dram, nwb, wpwb, l2=False,
                          dS_dram=None, dd_dram=None, opb=None, chb=None):
                qctr = [0]
                for bb in range(nwb):
                    wb_base = sched["seg_base"][(bb, 0)]
                    wb_len = sum(sched["L"][(bb, qx)] for qx in range(4))
                    gx_t = gxp.tile([128, (wb_len + 15) // 16], i16,
                                    tag="gx")
                    if wb_len:
                        nc.sync.dma_start(
                            gx_t[:],
                            gx_dram.ap()[:, wb_base // 16:
                                         (wb_base + wb_len) // 16])
                    if not l2:
                        no = opb[bb + 1] - opb[bb]
                        nch_b = chb[bb + 1] - chb[bb]
                        dS_t = dsp.tile([128, max(no, 1)], f32, tag="dS",
                                        name=f"dS{bb}")
                        if no:
                            nc.sync.dma_start(
                                dS_t[:], dS_dram.ap()[:, opb[bb]:opb[bb + 1]])
                        dd_t = dsp.tile([128, max(nch_b, 1)], f32, tag="dd",
                                        name=f"dd{bb}")
                        if nch_b:
                            nc.sync.dma_start(
                                dd_t[:], dd_dram.ap()[:, chb[bb]:chb[bb + 1]])
                    for qq in range(4):
                        Lseg = sched["L"][(bb, qq)]
                        if Lseg == 0:
                            continue
                        base = sched["seg_base"][(bb, qq)]
                        sb_l = base - wb_base
                        ncalls = (Lseg + CALL - 1) // CALL
                        stages = []
                        for k in range(ncalls):
                            cl = min(CALL, Lseg - CALL * k)
                            stg = stp.tile([128, CALL // 128, 128], bf16,
                                           tag="stg")
                            if contig_gather:
                                r0 = (base + CALL * k) % (QL - CALL)
                                nc.sync.dma_start(
                                    stg[:, :cl // 128, :],
                                    tabs[qq].ap()[r0:r0 + cl, :].rearrange(
                                        "(t p) f -> p t f", p=128))
                            else:
                                nc.gpsimd.dma_gather(
                                    stg[:, :cl // 128, :], tabs[qq].ap(),
                                    gx_t[:, (sb_l + CALL * k) // 16:
                                         (sb_l + CALL * k + cl) // 16],
                                    cl, cl, 128,
                                    queue_num=qctr[0] % 4)
                            qctr[0] += 1
                            stages.append(stg)
                        for op in sched["ops_by_seg"][(bb, qq)]:
                            _, _, jseg, jg, wv, oi, st_f, sp_f = op
                            k, jc = jseg // (CALL // 128), \
                                jseg % (CALL // 128)
                            wl = wv - bb * WPT
                            bank = banks[(bb % 2) * 3 + wl // 4]
                            bsl = bank[:, 128 * (wl % 4):128 * (wl % 4 + 1)]
                            S_t = ohp.tile([128, 128], bf16, tag="S")
                            if skip_mm:
                                continue
                            if l2:
                                nc.vector.tensor_scalar(
                                    S_t[:], iota_sb[:],
                                    dm2_sb[:, jg:jg + 1], float(-128.0 * wv),
                                    op0=mybir.AluOpType.subtract,
                                    op1=mybir.AluOpType.is_equal)
                                nc.tensor.matmul(
                                    bsl, lhsT=S_t[:], rhs=stages[k][:, jc, :],
                                    start=st_f, stop=sp_f)
                            else:
                                oloc = oi - opb[bb]
                                jloc = jg - chb[bb]
                                nc.vector.tensor_scalar(
                                    S_t[:], iota_sb[:],
                                    dS_t[:, oloc:oloc + 1],
                                    dd_t[:, jloc:jloc + 1],
                                    op0=mybir.AluOpType.is_equal,
                                    op1=mybir.AluOpType.mult)
                                nc.tensor.matmul(
                                    bsl, lhsT=stages[k][:, jc, :], rhs=S_t[:],
                                    start=st_f, stop=sp_f)
                    # flush (batched per block)
                    nwin = wpwb[bb]
                    if l2:
                        ob = flp.tile([128, WPT * 128], f32, tag="ow")
                    else:
                        ob = flp.tile([128, WPT * 128], bf16, tag="zw")
                    for wl in range(nwin):
                        wv = bb * WPT + wl
                        bank = banks[(bb % 2) * 3 + wl // 4]
                        bsl = bank[:, 128 * (wl % 4):128 * (wl % 4 + 1)]
                        osl = ob[:, 128 * wl:128 * (wl + 1)]
                        if l2:
                            if skip_mm:
                                nc.vector.tensor_copy(osl, b2_sb[:])
                            elif wv in sched["touched_w"]:
                                nc.vector.tensor_scalar(
                                    osl, bsl, dinv2_sb[:, wv:wv + 1], None,
                                    op0=mybir.AluOpType.mult)
                                nc.vector.tensor_tensor(
                                    osl, osl, b2_sb[:],
                                    op=mybir.AluOpType.add)
                            else:
                                nc.vector.tensor_copy(osl, b2_sb[:])
                        else:
                            if skip_mm:
                                nc.vector.memset(osl, 0.0)
                            elif wv in sched["touched_w"]:
                                nc.vector.tensor_scalar(
                                    osl, bsl, b1_sb[:, 0:1], 0.0,
                                    op0=mybir.AluOpType.add,
                                    op1=mybir.AluOpType.max)
                            else:
                                nc.vector.memset(osl, 0.0)
                    w0 = bb * WPT
                    if l2:
                        rows = min(S, 128 * (w0 + nwin)) - 128 * w0
                        nfull = rows // 128
                        if nfull:
                            nc.sync.dma_start(
                                out.ap()[128 * w0:128 * (w0 + nfull), :]
                                .rearrange("(t p) f -> p t f", p=128),
                                ob[:, :128 * nfull].rearrange(
                                    "p (t f) -> p t f", f=128))
                        rem = rows - 128 * nfull
                        if rem:
                            nc.sync.dma_start(
                                out.ap()[128 * (w0 + nfull):
                                         128 * (w0 + nfull) + rem, :],
                                ob[0:rem, 128 * nfull:128 * (nfull + 1)])
                    else:
                        nc.sync.dma_start(
                            zT.ap()[:, 128 * w0:128 * (w0 + nwin)],
                            ob[:, :128 * nwin])

            aggregate(s1, y1t, gx1, NWBG, wpwb_g, l2=False,
                      dS_dram=dS1, dd_dram=dd1, opb=op_base1, chb=ch_base1)

            # ---- phase C: y2 tables ----
            build_table(zT, W2_sb, y2t, bf16)

            # ---- phase D: layer-2 aggregation (own nodes) -> out ----
            aggregate(s2, y2t, gx2, NWB2, wpwb_2, l2=True)

            if loop_cm is not None:
                loop_cm.__exit__(None, None, None)

    nc.compile()
    return nc


def _make_in_maps(x, W1, b1, W2, b2, s1, s2, dinv):
    xT = np.zeros((128, NPAD), np.float32)
    xT[:, :N_NODES] = np.asarray(x, np.float32).T
    iota = np.broadcast_to(np.arange(128, dtype=np.float32),
                           (128, 128)).astype(np.float32)
    dinv_pad = np.zeros(NPAD, np.float32)
    dinv_pad[:N_NODES] = dinv
    dinvg = np.ascontiguousarray(dinv_pad.reshape(NWG, 128).T)
    gx1w = _wrap_idx(s1["gidx"])
    common = {
        "xT": xT, "W1": np.asarray(W1, np.float32),
        "W2b": np.asarray(W2, np.float32).astype(ml_dtypes.bfloat16),
        "b1c": np.asarray(b1, np.float32).reshape(128, 1),
        "b2b": np.broadcast_to(np.asarray(b2, np.float32),
                               (128, 128)).copy(),
        "dinvg": dinvg, "iotab": iota.astype(ml_dtypes.bfloat16),
        "gx1": gx1w, "dS1": s1["dstmS"], "dd1": s1["ddT"],
    }
    in_maps = []
    for c in range(N_CORES):
        d2 = np.zeros(NW2 * 128, np.float32)
        d2[:S] = dinv[S * c:S * (c + 1)]
        dinv2 = np.ascontiguousarray(d2.reshape(NW2, 128).T)
        in_maps.append({
            **common,
            "dinv2": dinv2,
            "gx2": _wrap_idx(s2["gidx"][c]),
            "dm2": s2["dstm"][c],
        })
    return in_maps


def kernel(x, edge_index, W1, b1, W2, b2):
    s1, s2, dinv = _prep(edge_index)
    nc = _build(s1, s2, repeat=int(os.environ.get("KERNEL_REPEAT", "1")))
    in_maps = _make_in_maps(x, W1, b1, W2, b2, s1, s2, dinv)
    res = run_bass_kernel_spmd(nc, in_maps, core_ids=list(range(N_CORES)))
    return np.concatenate([res.results[c]["out"] for c in range(N_CORES)], 0)


# revision 8
# speedup vs baseline: 6.8243x; 1.3901x over previous
"""GCN 2-layer encoder on 8 Trainium2 NeuronCores — zero-collective design.

Every core redundantly computes the cheap dense transforms for ALL nodes
(y1 = dinv*(x@W1), zT = relu-agg, y2 = dinv*(z@W2)) into LOCAL bf16 HBM
gather tables, so no AllGather is ever needed (collectives cost ~5ms each
on this runtime when awaited).  Layer-1 aggregation is replicated over all
100352 padded nodes (one-hot matmul segment-sum, gather-bandwidth bound);
layer-2 aggregation only covers the core's own 12500 dst nodes.

Messages are bf16 (halves gather bytes; PSUM accumulates f32).  The
one-hot S_t fuses dinv[dst] via tensor_scalar(is_equal, mult) with
per-op dstmS / per-chunk dd streamed from HBM per window-block.
"""
import os
import sys

sys.path.insert(0, "/opt/trn_rl_repo")
import numpy as np
import ml_dtypes

import concourse.tile as tile
from concourse import bacc, mybir, library_config
from concourse.bass_utils import run_bass_kernel_spmd

N_NODES = 100000
N_CORES = 8
S = N_NODES // N_CORES          # 12500 own nodes per core
D = 128
NPAD = 100352                   # 784 * 128
NWG = NPAD // 128               # 784 global dst windows (layer 1)
WPT = 12                        # windows per block (3 PSUM banks x 4)
NWBG = (NWG + WPT - 1) // WPT   # 66 global window blocks
QL = 25088                      # quarter length (196*128), int16-safe
QBG = [0, QL, 2 * QL, 3 * QL, 4 * QL]
NW2 = (S + 127) // 128          # 98 own dst windows (layer 2)
NWB2 = (NW2 + WPT - 1) // WPT   # 9
CALL = int(os.environ.get("KERNEL_CALL", "1024"))
f32 = mybir.dt.float32
bf16 = mybir.dt.bfloat16
i16 = mybir.dt.int16


def _pad128(n):
    return max(128, ((n + 127) // 128) * 128)


def _wrap_idx(gidx_flat):
    """[slots] int16 -> [128, slots/16] wrapped+replicated for dma_gather."""
    a = gidx_flat.reshape(-1, 16).T
    return np.tile(a, (8, 1)).copy()


def _sched_edges(src, dst, dinv, nwb, n_seg_q, dst_base=0, with_dd=True):
    """Common scheduling: segment edges by (window-block of dst, quarter of
    src), pad each segment to x128, emit chunk/op tables.

    dst here is already local (dst_base subtracted).  Returns dict with
    gidx [slots], dstm [128, nch] (local dst value per slot), per-op arrays,
    and op metadata list (b, q, k, jc, jl, wl, start, stop, oi).
    """
    w = dst // 128
    b = w // WPT
    q = src // QL
    if os.environ.get("SORT_DST"):
        order = np.lexsort((src, dst, q, b))
    else:
        order = np.lexsort((src, w, q, b))
    so, do, qo, bo = src[order], dst[order], q[order], b[order]
    segkey = bo * 4 + qo
    n = len(so)
    bounds = np.flatnonzero(np.diff(segkey)) + 1
    starts = np.concatenate([[0], bounds])
    ends = np.concatenate([bounds, [n]])
    seg_edges = {int(segkey[s0]): (s0, e0) for s0, e0 in zip(starts, ends)}

    seg_list = [(bb, qq) for bb in range(nwb) for qq in range(4)]
    L = {}
    for (bb, qq) in seg_list:
        k = bb * 4 + qq
        ln = seg_edges[k][1] - seg_edges[k][0] if k in seg_edges else 0
        L[(bb, qq)] = _pad128(ln) if ln else 0
    tot = sum(L.values())
    gidx = np.zeros(tot, np.int16)
    dstv = np.full(tot, -1.0e6, np.float64)
    ddv = np.zeros(tot, np.float64)
    seg_base = {}
    off = 0
    for (bb, qq) in seg_list:
        seg_base[(bb, qq)] = off
        k = bb * 4 + qq
        if k in seg_edges:
            s0, e0 = seg_edges[k]
            m = e0 - s0
            gidx[off:off + m] = (so[s0:e0] - QBG[qo[s0]]).astype(np.int16)
            dstv[off:off + m] = do[s0:e0]
            if with_dd:
                ddv[off:off + m] = dinv[do[s0:e0] + dst_base]
        off += L[(bb, qq)]

    nch = tot // 128
    slot_ch = np.arange(tot) // 128
    real = dstv >= 0
    wl_slot = np.where(real, (dstv // 128).astype(np.int64), 0)
    # ops: unique (chunk, window) among real slots
    opkey = slot_ch * NWG + wl_slot
    op_keys = np.unique(opkey[real])
    n_ops = len(op_keys)
    op_of_slot = np.searchsorted(op_keys, opkey)
    # per-op dstm (dst - 128*w for slots in that op's window, else -1000)
    dstmS = np.full((n_ops, 128), -1000.0, np.float32)
    sl = np.flatnonzero(real)
    dstmS[op_of_slot[sl], sl % 128] = (dstv[sl] - 128.0 * wl_slot[sl])
    dstmS = np.ascontiguousarray(dstmS.T)             # [128, n_ops]
    ddT = np.zeros((nch, 128), np.float32)
    ddT[slot_ch[sl], sl % 128] = ddv[sl]
    ddT = np.ascontiguousarray(ddT.T)                 # [128, nch]
    dstmC = np.full((nch, 128), -1.0e6, np.float32)
    dstmC[slot_ch[sl], sl % 128] = dstv[sl]
    dstmC = np.ascontiguousarray(dstmC.T)             # [128, nch]

    # op metadata in emission order (seg-major)
    op_ch = op_keys // NWG
    op_w = op_keys % NWG
    ops = []
    first_bk, last_bk = {}, {}
    for oi in range(n_ops):
        j = int(op_ch[oi])
        wv = int(op_w[oi])
        base = j * 128
        # find seg of this chunk
        bb = int(wv // WPT)
        qq = None
        for qx in range(4):
            sb = seg_base[(bb, qx)]
            if sb <= base < sb + L[(bb, qx)]:
                qq = qx
                break
        assert qq is not None, (j, wv, bb)
        jseg = (base - seg_base[(bb, qq)]) // 128
        ops.append([bb, qq, jseg, j, wv, oi, False, False])
        bk = (bb, (wv - bb * WPT) // 4)
        if bk not in first_bk:
            first_bk[bk] = len(ops) - 1
        last_bk[bk] = len(ops) - 1
    for i in first_bk.values():
        ops[i][6] = True
    for i in last_bk.values():
        ops[i][7] = True
    ops_by_seg = {sk: [] for sk in seg_list}
    for op in ops:
        ops_by_seg[(op[0], op[1])].append(op)
    # interleave ops across windows within each segment so consecutive PE
    # matmuls hit different PSUM banks/addresses (avoids accumulation-drain
    # serialization on same-window chunk runs)
    if not os.environ.get("NO_INTERLEAVE"):
        for sk in seg_list:
            lst = ops_by_seg[sk]
            cnt = {}
            keyed = []
            for op in lst:
                r = cnt.get(op[4], 0)
                cnt[op[4]] = r + 1
                wl = op[4] - op[0] * WPT
                keyed.append((r, wl % 4, wl // 4, op))
            lst2 = [t[3] for t in sorted(keyed,
                                         key=lambda t: (t[0], t[1], t[2]))]
            ops_by_seg[sk] = lst2
        # recompute start/stop flags in final emission order
        flat = []
        for sk in seg_list:
            flat.extend(ops_by_seg[sk])
        for op in flat:
            op[6] = op[7] = False
        first_bk, last_bk = {}, {}
        for i, op in enumerate(flat):
            bk = (op[0], (op[4] - op[0] * WPT) // 4)
            if bk not in first_bk:
                first_bk[bk] = i
            last_bk[bk] = i
        for i in first_bk.values():
            flat[i][6] = True
        for i in last_bk.values():
            flat[i][7] = True
    touched_w = set(int(x) for x in np.unique(op_w))
    return {
        "L": L, "seg_base": seg_base, "seg_list": seg_list,
        "tot": tot, "nch": nch, "n_ops": n_ops,
        "gidx": gidx, "dstmS": dstmS, "ddT": ddT, "dstmC": dstmC,
        "ops_by_seg": ops_by_seg, "touched_w": touched_w,
    }


def _prep(edge_index):
    """Returns (sched_l1, scheds_l2, dinv). sched_l1 is common to all
    cores; scheds_l2 is per-core with common shapes/op-structure."""
    src = np.asarray(edge_index[0], dtype=np.int64)
    dst = np.asarray(edge_index[1], dtype=np.int64)
    deg = (np.bincount(dst, minlength=N_NODES) + 1).astype(np.float64)
    dinv = (1.0 / np.sqrt(deg)).astype(np.float64)

    loop = np.arange(N_NODES, dtype=np.int64)
    src_all = np.concatenate([src, loop])
    dst_all = np.concatenate([dst, loop])

    s1 = _sched_edges(src_all, dst_all, dinv, NWBG, 4)

    # layer 2: per-core own dst, common padding/op structure
    core = dst_all // S
    percore = []
    for c in range(N_CORES):
        m = core == c
        percore.append(_sched_edges(src_all[m], dst_all[m] - c * S, dinv,
                                    NWB2, 4, dst_base=c * S, with_dd=False))
    # unify L across cores, rebuild with common layout
    Lc = {}
    for sk in percore[0]["L"]:
        Lc[sk] = max(p["L"][sk] for p in percore)
    totc = sum(Lc.values())
    nchc = totc // 128
    seg_base = {}
    off = 0
    for sk in percore[0]["seg_list"]:
        seg_base[sk] = off
        off += Lc[sk]
    # re-embed each core's slot arrays into the common layout
    gidx2 = np.zeros((N_CORES, totc), np.int16)
    dstm2 = np.full((N_CORES, 128, nchc), -1.0e6, np.float32)
    opset = set()
    for c in range(N_CORES):
        p = percore[c]
        for sk in p["seg_list"]:
            sb_c, sb_u = p["seg_base"][sk], seg_base[sk]
            ln = p["L"][sk]
            if ln == 0:
                continue
            gidx2[c, sb_u:sb_u + ln] = p["gidx"][sb_c:sb_c + ln]
            for j in range(ln // 128):
                dstm2[c, :, (sb_u // 128) + j] = \
                    p["dstmC"][:, (sb_c // 128) + j]
        # collect op (chunk_global_unified, window) pairs
        for sk in p["seg_list"]:
            for op in p["ops_by_seg"][sk]:
                bb, qq, jseg, _, wv = op[0], op[1], op[2], op[3], op[4]
                ju = (seg_base[(bb, qq)] // 128) + jseg
                opset.add((bb, qq, jseg, ju, wv))
    ops2 = sorted(opset, key=lambda t: (t[0], t[1], t[2], t[4]))
    ops_list = []
    first_bk, last_bk = {}, {}
    for (bb, qq, jseg, ju, wv) in ops2:
        ops_list.append([bb, qq, jseg, ju, wv, None, False, False])
        bk = (bb, (wv - bb * WPT) // 4)
        if bk not in first_bk:
            first_bk[bk] = len(ops_list) - 1
        last_bk[bk] = len(ops_list) - 1
    for i in first_bk.values():
        ops_list[i][6] = True
    for i in last_bk.values():
        ops_list[i][7] = True
    ops_by_seg = {sk: [] for sk in percore[0]["seg_list"]}
    for op in ops_list:
        ops_by_seg[(op[0], op[1])].append(op)
    touched = set()
    for c in range(N_CORES):
        touched |= percore[c]["touched_w"]
    sched_l2 = {
        "L": Lc, "seg_base": seg_base, "tot": totc, "nch": nchc,
        "seg_list": percore[0]["seg_list"],
        "gidx": gidx2, "dstm": dstm2, "ops_by_seg": ops_by_seg,
        "touched_w": touched,
    }
    return s1, sched_l2, dinv.astype(np.float32)


def _build(s1, s2, repeat=1, skip_mm=False, contig_gather=False,
           skip_dve=False, ohb=4):
    nc = bacc.Bacc("TRN2", target_bir_lowering=False, debug=False,
                   num_devices=N_CORES, num_swdge_queues=4)
    xT = nc.dram_tensor("xT", [128, NPAD], f32, kind="ExternalInput")
    W1 = nc.dram_tensor("W1", [128, 128], f32, kind="ExternalInput")
    W2b = nc.dram_tensor("W2b", [128, 128], bf16, kind="ExternalInput")
    b1c = nc.dram_tensor("b1c", [128, 1], f32, kind="ExternalInput")
    b2b = nc.dram_tensor("b2b", [128, 128], f32, kind="ExternalInput")
    dinvg = nc.dram_tensor("dinvg", [128, NWG], f32, kind="ExternalInput")
    dinv2 = nc.dram_tensor("dinv2", [128, NW2], f32, kind="ExternalInput")
    iotab = nc.dram_tensor("iotab", [128, 128], bf16, kind="ExternalInput")
    gx1 = nc.dram_tensor("gx1", [128, s1["tot"] // 16], i16,
                         kind="ExternalInput")
    dS1 = nc.dram_tensor("dS1", [128, s1["n_ops"]], f32,
                         kind="ExternalInput")
    dd1 = nc.dram_tensor("dd1", [128, s1["nch"]], f32, kind="ExternalInput")
    gx2 = nc.dram_tensor("gx2", [128, s2["tot"] // 16], i16,
                         kind="ExternalInput")
    dm2 = nc.dram_tensor("dm2", [128, s2["nch"]], f32, kind="ExternalInput")
    out = nc.dram_tensor("out", [S, D], f32, kind="ExternalOutput")

    y1t = [nc.dram_tensor(f"y1t{q}", [QL, 128], bf16) for q in range(4)]
    y2t = [nc.dram_tensor(f"y2t{q}", [QL, 128], bf16) for q in range(4)]
    zT = nc.dram_tensor("zT", [128, NPAD], bf16)

    # per-wb column ranges of dS1/dd1 for streaming
    op_base1, ch_base1 = [], []
    nop_acc = 0
    for bb in range(NWBG):
        op_base1.append(nop_acc)
        nop_acc += sum(len(s1["ops_by_seg"][(bb, qq)]) for qq in range(4))
    op_base1.append(nop_acc)
    assert nop_acc == s1["n_ops"]
    for bb in range(NWBG):
        ch_base1.append(s1["seg_base"][(bb, 0)] // 128)
    ch_base1.append(s1["nch"])

    wpwb_g = [min(NWG - bb * WPT, WPT) for bb in range(NWBG)]
    wpwb_2 = [min(NW2 - bb * WPT, WPT) for bb in range(NWB2)]

    with tile.TileContext(nc) as tc:
        with (
            tc.tile_pool(name="cst", bufs=1) as cst,
            tc.tile_pool(name="blk", bufs=4) as blkp,
            tc.tile_pool(name="ps", bufs=2, space="PSUM") as php,
            tc.tile_pool(name="st", bufs=8) as stp,
            tc.tile_pool(name="gxp", bufs=3) as gxp,
            tc.tile_pool(name="dsp", bufs=2) as dsp,
            tc.tile_pool(name="oh", bufs=ohb) as ohp,
            tc.tile_pool(name="bank", bufs=1, space="PSUM") as bankp,
            tc.tile_pool(name="fl", bufs=4) as flp,
        ):
            nc.gpsimd.load_library(library_config.mlp)

            W1_sb = cst.tile([128, 128], f32, tag="W1")
            W2_sb = cst.tile([128, 128], bf16, tag="W2")
            b1_sb = cst.tile([128, 1], f32, tag="b1")
            b2_sb = cst.tile([128, 128], f32, tag="b2")
            dinvg_sb = cst.tile([128, NWG], f32, tag="dg")
            dinv2_sb = cst.tile([128, NW2], f32, tag="d2")
            iota_sb = cst.tile([128, 128], bf16, tag="iota")
            dm2_sb = cst.tile([128, s2["nch"]], f32, tag="dm2")
            nc.sync.dma_start(W1_sb[:], W1[:])
            nc.sync.dma_start(W2_sb[:], W2b[:])
            nc.sync.dma_start(b1_sb[:], b1c[:])
            nc.sync.dma_start(b2_sb[:], b2b[:])
            nc.sync.dma_start(dinvg_sb[:], dinvg[:])
            nc.sync.dma_start(dinv2_sb[:], dinv2[:])
            nc.sync.dma_start(iota_sb[:], iotab[:])
            nc.sync.dma_start(dm2_sb[:], dm2[:])
            banks = [bankp.tile([128, 512], f32, tag=f"bk{i}",
                                name=f"bank{i}") for i in range(6)]

            loop_cm = tc.For_i(0, repeat, 1) if repeat > 1 else None
            if loop_cm is not None:
                loop_cm.__enter__()

            # ---- phase A: y1 tables (all nodes) ----
            def build_table(src_dram, w_sb, tabs, src_dt):
                G = 8
                for g0 in range(0, NWG, G):
                    ng = min(G, NWG - g0)
                    xt = blkp.tile([128, G * 128], src_dt, tag="xt")
                    nc.sync.dma_start(
                        xt[:, :ng * 128],
                        src_dram.ap()[:, 128 * g0:128 * (g0 + ng)])
                    yb = blkp.tile([128, G * 128], bf16, tag="yb")
                    for i in range(ng):
                        t = g0 + i
                        ps = php.tile([128, 128], f32, tag="php")
                        nc.tensor.matmul(
                            ps[:], lhsT=xt[:, 128 * i:128 * (i + 1)],
                            rhs=w_sb[:], start=True, stop=True)
                        nc.vector.tensor_scalar(
                            yb[:, 128 * i:128 * (i + 1)], ps[:],
                            dinvg_sb[:, t:t + 1], None,
                            op0=mybir.AluOpType.mult)
                    # DMA out, split at quarter boundaries
                    i = 0
                    while i < ng:
                        t = g0 + i
                        q = (128 * t) // QL
                        nblk = min(ng - i, (QBG[q + 1] - 128 * t) // 128)
                        r0 = 128 * t - QBG[q]
                        nc.sync.dma_start(
                            tabs[q].ap()[r0:r0 + 128 * nblk, :].rearrange(
                                "(t p) f -> p t f", p=128),
                            yb[:, 128 * i:128 * (i + nblk)].rearrange(
                                "p (t f) -> p t f", f=128))
                        i += nblk

            build_table(xT, W1_sb, y1t, f32)

            # ---- phase B: layer-1 aggregation over all nodes -> zT ----
            def aggregate(sched, tabs, gx_dram, nwb, wpwb, l2=False,
                          dS_dram=None, dd_dram=None, opb=None, chb=None):
                qctr = [0]
                for bb in range(nwb):
                    wb_base = sched["seg_base"][(bb, 0)]
                    wb_len = sum(sched["L"][(bb, qx)] for qx in range(4))
                    gx_t = gxp.tile([128, (wb_len + 15) // 16], i16,
                                    tag="gx")
                    if wb_len:
                        nc.sync.dma_start(
                            gx_t[:],
                            gx_dram.ap()[:, wb_base // 16:
                                         (wb_base + wb_len) // 16])
                    if not l2:
                        no = opb[bb + 1] - opb[bb]
                        nch_b = chb[bb + 1] - chb[bb]
                        dS_t = dsp.tile([128, max(no, 1)], f32, tag="dS",
                                        name=f"dS{bb}")
                        if no:
                            nc.sync.dma_start(
                                dS_t[:], dS_dram.ap()[:, opb[bb]:opb[bb + 1]])
                        dd_t = dsp.tile([128, max(nch_b, 1)], f32, tag="dd",
                                        name=f"dd{bb}")
                        if nch_b:
                            nc.sync.dma_start(
                                dd_t[:], dd_dram.ap()[:, chb[bb]:chb[bb + 1]])
                    for qq in range(4):
                        Lseg = sched["L"][(bb, qq)]
                        if Lseg == 0:
                            continue
                        base = sched["seg_base"][(bb, qq)]
                        sb_l = base - wb_base
                        ncalls = (Lseg + CALL - 1) // CALL
                        stages = []
                        for k in range(ncalls):
                            cl = min(CALL, Lseg - CALL * k)
                            stg = stp.tile([128, CALL // 128, 128], bf16,
                                           tag="stg")
                            if contig_gather:
                                r0 = (base + CALL * k) % (QL - CALL)
                                nc.sync.dma_start(
                                    stg[:, :cl // 128, :],
                                    tabs[qq].ap()[r0:r0 + cl, :].rearrange(
                                        "(t p) f -> p t f", p=128))
                            else:
                                nc.gpsimd.dma_gather(
                                    stg[:, :cl // 128, :], tabs[qq].ap(),
                                    gx_t[:, (sb_l + CALL * k) // 16:
                                         (sb_l + CALL * k + cl) // 16],
                                    cl, cl, 128,
                                    queue_num=qctr[0] % 4)
                            qctr[0] += 1
                            stages.append(stg)
                        for op in sched["ops_by_seg"][(bb, qq)]:
                            _, _, jseg, jg, wv, oi, st_f, sp_f = op
                            k, jc = jseg // (CALL // 128), \
                                jseg % (CALL // 128)
                            wl = wv - bb * WPT
                            bank = banks[(bb % 2) * 3 + wl // 4]
                            bsl = bank[:, 128 * (wl % 4):128 * (wl % 4 + 1)]
                            S_t = ohp.tile([128, 128], bf16, tag="S")
                            if skip_mm:
                                continue
                            if l2:
                                nc.vector.tensor_scalar(
                                    S_t[:], iota_sb[:],
                                    dm2_sb[:, jg:jg + 1], float(-128.0 * wv),
                                    op0=mybir.AluOpType.subtract,
                                    op1=mybir.AluOpType.is_equal)
                                nc.tensor.matmul(
                                    bsl, lhsT=S_t[:], rhs=stages[k][:, jc, :],
                                    start=st_f, stop=sp_f)
                            else:
                                oloc = oi - opb[bb]
                                jloc = jg - chb[bb]
                                nc.vector.tensor_scalar(
                                    S_t[:], iota_sb[:],
                                    dS_t[:, oloc:oloc + 1],
                                    dd_t[:, jloc:jloc + 1],
                                    op0=mybir.AluOpType.is_equal,
                                    op1=mybir.AluOpType.mult)
                                nc.tensor.matmul(
                                    bsl, lhsT=stages[k][:, jc, :], rhs=S_t[:],
                                    start=st_f, stop=sp_f)
                    # flush (batched per block)
                    nwin = wpwb[bb]
                    if l2:
                        ob = flp.tile([128, WPT * 128], f32, tag="ow")
                    else:
                        ob = flp.tile([128, WPT * 128], bf16, tag="zw")
                    for wl in range(nwin):
                        wv = bb * WPT + wl
                        bank = banks[(bb % 2) * 3 + wl // 4]
                        bsl = bank[:, 128 * (wl % 4):128 * (wl % 4 + 1)]
                        osl = ob[:, 128 * wl:128 * (wl + 1)]
                        if l2:
                            if skip_mm:
                                nc.vector.tensor_copy(osl, b2_sb[:])
                            elif wv in sched["touched_w"]:
                                nc.vector.tensor_scalar(
                                    osl, bsl, dinv2_sb[:, wv:wv + 1], None,
                                    op0=mybir.AluOpType.mult)
                                nc.vector.tensor_tensor(
                                    osl, osl, b2_sb[:],
                                    op=mybir.AluOpType.add)
                            else:
                                nc.vector.tensor_copy(osl, b2_sb[:])
                        else:
                            if skip_mm:
                                nc.vector.memset(osl, 0.0)
                            elif wv in sched["touched_w"]:
                                nc.vector.tensor_scalar(
                                    osl, bsl, b1_sb[:, 0:1], 0.0,
                                    op0=mybir.AluOpType.add,
                                    op1=mybir.AluOpType.max)
                            else:
                                nc.vector.memset(osl, 0.0)
                    w0 = bb * WPT
                    if l2:
                        rows = min(S, 128 * (w0 + nwin)) - 128 * w0
                        nfull = rows // 128
                        if nfull:
                            nc.sync.dma_start(
                                out.ap()[128 * w0:128 * (w0 + nfull), :]
                                .rearrange("(t p) f -> p t f", p=128),
                                ob[:, :128 * nfull].rearrange(
                                    "p (t f) -> p t f", f=128))
                        rem = rows - 128 * nfull
                        if rem:
                            nc.sync.dma_start(
                                out.ap()[128 * (w0 + nfull):
                                         128 * (w0 + nfull) + rem, :],
                                ob[0:rem, 128 * nfull:128 * (nfull + 1)])
                    else:
                        nc.sync.dma_start(
                            zT.ap()[:, 128 * w0:128 * (w0 + nwin)],
                            ob[:, :128 * nwin])

            aggregate(s1, y1t, gx1, NWBG, wpwb_g, l2=False,
                      dS_dram=dS1, dd_dram=dd1, opb=op_base1, chb=ch_base1)

            # ---- phase C: y2 tables ----
            build_table(zT, W2_sb, y2t, bf16)

            # ---- phase D: layer-2 aggregation (own nodes) -> out ----
            aggregate(s2, y2t, gx2, NWB2, wpwb_2, l2=True)

            if loop_cm is not None:
                loop_cm.__exit__(None, None, None)

    nc.compile()
    return nc


def _make_in_maps(x, W1, b1, W2, b2, s1, s2, dinv):
    xT = np.zeros((128, NPAD), np.float32)
    xT[:, :N_NODES] = np.asarray(x, np.float32).T
    iota = np.broadcast_to(np.arange(128, dtype=np.float32),
                           (128, 128)).astype(np.float32)
    dinv_pad = np.zeros(NPAD, np.float32)
    dinv_pad[:N_NODES] = dinv
    dinvg = np.ascontiguousarray(dinv_pad.reshape(NWG, 128).T)
    gx1w = _wrap_idx(s1["gidx"])
    common = {
        "xT": xT, "W1": np.asarray(W1, np.float32),
        "W2b": np.asarray(W2, np.float32).astype(ml_dtypes.bfloat16),
        "b1c": np.asarray(b1, np.float32).reshape(128, 1),
        "b2b": np.broadcast_to(np.asarray(b2, np.float32),
                               (128, 128)).copy(),
        "dinvg": dinvg, "iotab": iota.astype(ml_dtypes.bfloat16),
        "gx1": gx1w, "dS1": s1["dstmS"], "dd1": s1["ddT"],
    }
    in_maps = []
    for c in range(N_CORES):
        d2 = np.zeros(NW2 * 128, np.float32)
        d2[:S] = dinv[S * c:S * (c + 1)]
        dinv2 = np.ascontiguousarray(d2.reshape(NW2, 128).T)
        in_maps.append({
            **common,
            "dinv2": dinv2,
            "gx2": _wrap_idx(s2["gidx"][c]),
            "dm2": s2["dstm"][c],
        })
    return in_maps


def kernel(x, edge_index, W1, b1, W2, b2):
    s1, s2, dinv = _prep(edge_index)
    nc = _build(s1, s2, repeat=int(os.environ.get("KERNEL_REPEAT", "1")))
    in_maps = _make_in_maps(x, W1, b1, W2, b2, s1, s2, dinv)
    res = run_bass_kernel_spmd(nc, in_maps, core_ids=list(range(N_CORES)))
    return np.concatenate([res.results[c]["out"] for c in range(N_CORES)], 0)
